# revision 17
# baseline (speedup 1.0000x reference)
import numpy as np
import concourse.bass as bass
import concourse.tile as tile
from concourse import mybir
from concourse.bass_utils import run_bass_kernel_spmd

P = 128
S = 2048
D = 512
U = 1024
NS = S // P      # 16 s-tiles
ND = D // P      # 4 d-blocks
NEG = -60000.0
EPS = 1e-6
EBIAS = -10.0    # uniform exp bias; cancels in the Z normalization


def _patched_drain_and_barrier(self, tick_clock, wait_clock):
    nc = self.nc
    probe = nc.sync.nop(nofuse=True, hint="drain_waits_probe")
    wait_clock.add_sem_waits(probe.ins, tile.ScopedClock({None: tick_clock.global_clock}))
    si = probe.ins.sync_info
    waits = list(si.on_wait) if si is not None else []
    assert self.sems is not None
    handles = {h.name: h for h in self.sems.allocated().values()}
    if len(waits) > 1:
        import bass_rust
        probe.ins.sync_info = bass_rust.SyncInfo(on_wait=waits[:1], on_update=[])
        for w in waits[1:]:
            h = handles.get(w.ant_name)
            assert h is not None, (w.ant_name, list(handles))
            nc.sync.wait_ge(h, w.wait_value)
    nc.sync.drain()
    nc.all_engine_barrier()
    popped = nc._tile_sem_poison_stack.pop()
    assert popped is self._sem_poison
    nc.clear_and_free_semaphores(list(self.sems.allocated().values()))
    nc.all_engine_barrier()


tile.TileContext._drain_and_barrier = _patched_drain_and_barrier

# The walrus backend in this toolchain rejects instructions carrying more
# than one semaphore wait ("Too many sync wait commands"). Split excess
# waits onto single-wait NoOp carriers on the same engine, which execute
# in order ahead of the real instruction.
_MAXW = 1
_orig_lower_ordered = tile.TileContext._lower_ordered_insts


def _patched_lower_ordered(self, ordered):
    nc = self.nc
    for insts in ordered.values():
        out = []
        for inst in insts:
            si = getattr(inst, "sync_info", None)
            eng = getattr(inst, "engine", None)
            if (si is not None and si.on_wait and len(si.on_wait) > _MAXW
                    and eng is not None
                    and not type(inst).__name__.startswith("BassTile")):
                waits = list(si.on_wait)
                for w in waits[:-_MAXW]:
                    out.append(mybir.InstNoOp(
                        name=nc.get_next_instruction_name(),
                        engine=eng,
                        ins=[],
                        outs=[],
                        bass_nofuse=True,
                        sync_info=mybir.SyncInfo(on_wait=[w], on_update=[]),
                    ))
                inst.sync_info = mybir.SyncInfo(
                    on_wait=waits[-_MAXW:], on_update=list(si.on_update))
            out.append(inst)
        insts[:] = out
    return _orig_lower_ordered(self, ordered)


tile.TileContext._lower_ordered_insts = _patched_lower_ordered

f32 = mybir.dt.float32
f16 = mybir.dt.float16
bf16 = mybir.dt.bfloat16


def _build():
    nc = bass.Bass()
    x_ext = nc.declare_dram_parameter("x", [S, D], f32, isOutput=False)
    ck_ext = nc.declare_dram_parameter("ck", [P, 2 * ND + 2], f32, isOutput=False)
    idf_ext = nc.declare_dram_parameter("idf", [P, P], f16, isOutput=False)
    idb_ext = nc.declare_dram_parameter("idb", [P, P], bf16, isOutput=False)
    msk_ext = nc.declare_dram_parameter("msk", [P, 4 * D], f16, isOutput=False)
    wa_ext = nc.declare_dram_parameter("wa", [P, 2 * ND * D], f16, isOutput=False)
    wg_ext = nc.declare_dram_parameter("wg", [P, 2 * ND * D], f16, isOutput=False)
    out_ext = nc.declare_dram_parameter("out", [S, D], f16, isOutput=True)

    with tile.TileContext(nc) as tc:
        with tc.tile_pool(name="const", bufs=1) as cp, \
             tc.tile_pool(name="xnt", bufs=1) as xp, \
             tc.tile_pool(name="bmat", bufs=1) as bp, \
             tc.tile_pool(name="vo", bufs=1) as vp, \
             tc.tile_pool(name="ln", bufs=2) as lp, \
             tc.tile_pool(name="xhp", bufs=4) as xhp, \
             tc.tile_pool(name="xd", bufs=8) as xdp, \
             tc.tile_pool(name="att", bufs=2) as ap_, \
             tc.tile_pool(name="st", bufs=2) as sp, \
             tc.tile_pool(name="oac", bufs=1) as op, \
             tc.tile_pool(name="outp", bufs=2) as up, \
             tc.tile_pool(name="mm", bufs=2, space="PSUM") as mmp, \
             tc.tile_pool(name="sc", bufs=2, space="PSUM") as scp, \
             tc.tile_pool(name="pv", bufs=1, space="PSUM") as pvp, \
             tc.tile_pool(name="tr", bufs=2, space="PSUM") as trp, \
             tc.tile_pool(name="trx", bufs=1, space="PSUM") as trxp:

            ident = cp.tile([P, P], f16, tag="ident", name="ident")
            identb = cp.tile([P, P], bf16, tag="identb", name="identb")
            ckt = cp.tile([P, 2 * ND + 2], f32, tag="ckt", name="ckt")
            mask = cp.tile([P, 4 * D], f16, tag="mask", name="mask")
            waT = cp.tile([P, 2 * ND * D], f16, tag="waT", name="waT")
            wgT = cp.tile([P, 2 * ND * D], f16, tag="wgT", name="wgT")
            nc.scalar.dma_start(out=ident[:], in_=idf_ext[:, :])
            nc.scalar.dma_start(out=identb[:], in_=idb_ext[:, :])
            nc.scalar.dma_start(out=ckt[:], in_=ck_ext[:, :])
            eps = ckt[:, 2 * ND:2 * ND + 1]
            ebias = ckt[:, 2 * ND + 1:2 * ND + 2]

            xnT = [xp.tile([P, S], f16, tag=f"xnt{j}", name=f"xnt{j}") for j in range(ND)]
            BT = [[bp.tile([P, S], f16, tag=f"bt{h}_{j}", name=f"bt{h}_{j}")
                   for j in range(ND)] for h in range(2)]
            Vo = [[vp.tile([P, D], bf16, tag=f"vo{h}_{t}", name=f"vo{h}_{t}")
                   for t in range(NS)] for h in range(2)]
            oacc = [op.tile([P, D], f32, tag=f"oacc{i}", name=f"oacc{i}")
                    for i in range(NS)]

            dmaq = [nc.sync, nc.gpsimd, nc.scalar]
            xq = {0: 0, 1: 1, 2: 2, 3: 0}
            for _t in range(4, NS):
                xq[_t] = [0, 1, 2][_t % 3]

            def emit_ln_group(g):
                tiles = list(range(4 * g, 4 * g + 4))
                xts, mvs, sds, xhs = [], [], [], []
                for i in tiles:
                    xt = xdp.tile([P, D], f32, tag="x", name="xt")
                    dmaq[xq[i]].dma_start(out=xt[:], in_=x_ext[i * P:(i + 1) * P, :])
                    xts.append(xt)
                    stats = lp.tile([P, 6], f32, tag=f"bs{i % 4}", name="bs")
                    nc.vector.bn_stats(out=stats[:], in_=xt[:])
                    mv = lp.tile([P, 2], f32, tag=f"mv{i % 4}", name="mv")
                    nc.vector.bn_aggr(out=mv[:], in_=stats[:])
                    mvs.append(mv)
                for k in range(4):
                    sd = lp.tile([P, 1], f32, tag=f"sd{k}", name="sd")
                    nc.scalar.activation(out=sd[:], in_=mvs[k][:, 1:2],
                                         func=mybir.ActivationFunctionType.Sqrt,
                                         bias=eps, scale=1.0, alpha=0.0)
                    sds.append(sd)
                for k in range(4):
                    nc.vector.reciprocal(out=sds[k][:], in_=sds[k][:])
                for k in range(4):
                    xh = xhp.tile([P, D], f16, tag=f"xh{k}", name="xh")
                    nc.vector.tensor_scalar(out=xh[:], in0=xts[k][:],
                                            scalar1=mvs[k][:, 0:1], scalar2=sds[k][:],
                                            op0=mybir.AluOpType.subtract,
                                            op1=mybir.AluOpType.mult)
                    xhs.append(xh)
                if g == 0:
                    # per-tile batches: first transpose starts at xh0
                    for kk in range(4):
                        tp = trxp.tile([P, 512], f16, tag="trx", name="tpx")
                        for j in range(ND):
                            nc.tensor.transpose(tp[:, j * P:(j + 1) * P],
                                                xhs[kk][:, j * P:(j + 1) * P],
                                                ident[:])
                        for j in range(ND):
                            nc.any.tensor_copy(
                                out=xnT[j][:, (4 * g + kk) * P:(4 * g + kk + 1) * P],
                                in_=tp[:, j * P:(j + 1) * P])
                else:
                    for j in range(ND):
                        tp = trxp.tile([P, 512], f16, tag="trx", name="tpx")
                        for kk in range(4):
                            nc.tensor.transpose(tp[:, kk * P:(kk + 1) * P],
                                                xhs[kk][:, j * P:(j + 1) * P], ident[:])
                        nc.any.tensor_copy(out=xnT[j][:, g * 512:(g + 1) * 512], in_=tp[:])

            def emit_b_sl(h, sl):
                for jp in range(ND):
                    mm = mmp.tile([P, 512], f32, tag="mm", name="mm")
                    for j in range(ND):
                        nc.tensor.matmul(mm[:],
                                         waT[:, (h * ND + j) * D + jp * P:
                                             (h * ND + j) * D + (jp + 1) * P],
                                         xnT[j][:, sl * 512:(sl + 1) * 512],
                                         start=(j == 0), stop=(j == ND - 1))
                    nc.any.tensor_scalar_add(out=BT[h][jp][:, sl * 512:(sl + 1) * 512],
                                             in0=mm[:],
                                             scalar1=ckt[:, h * ND + jp:h * ND + jp + 1])

            def emit_vo_tile(h, t):
                mm = mmp.tile([P, 512], f32, tag="mm", name="mm")
                for j in range(ND):
                    nc.tensor.matmul(mm[:],
                                     xnT[j][:, t * P:(t + 1) * P],
                                     wgT[:, (h * ND + j) * D:(h * ND + j + 1) * D],
                                     start=(j == 0), stop=(j == ND - 1))
                nc.any.tensor_copy(out=Vo[h][t][:], in_=mm[:])

            def emit_scores(h, i):
                nch = i // 4 + 1
                Pt = ap_.tile([P, S], bf16, tag="P", name="Pt")
                rsum = sp.tile([P, 4], f32, tag="rsum", name="rsum")
                for c in range(nch):
                    diag = (c == i // 4)
                    w = (i % 4 + 1) * P if diag else 512
                    sc = scp.tile([P, 512], f32, tag="sc", name="sc")
                    for j in range(ND):
                        nc.tensor.matmul(sc[:, 0:w],
                                         BT[h][j][:, i * P:(i + 1) * P],
                                         xnT[j][:, c * 512:c * 512 + w],
                                         start=(j == 0),
                                         stop=(j == ND - 1) and not diag)
                    if diag:
                        m = i % 4
                        nc.tensor.matmul(sc[:, 0:w], ident[:],
                                         mask[:, m * 512:m * 512 + w],
                                         start=False, stop=True)
                    nc.scalar.activation(out=Pt[:, c * 512:c * 512 + w], in_=sc[:, 0:w],
                                         func=mybir.ActivationFunctionType.Exp,
                                         bias=ebias, scale=1.0,
                                         accum_out=rsum[:, c:c + 1])
                tot = sp.tile([P, 1], f32, tag="tot", name="tot")
                nc.vector.reduce_sum(out=tot[:], in_=rsum[:, 0:nch],
                                     axis=mybir.AxisListType.X)
                zr = sp.tile([P, 1], f32, tag="zr", name="zr")
                nc.vector.reciprocal(out=zr[:], in_=tot[:])
                return Pt, zr

            def emit_tail(h, i, Pt, zr, final=False):
                # transpose probs blocks 0..i, batched 4 per PSUM tile
                pt = ap_.tile([P, S], bf16, tag="pt", name="pt")
                nt = i + 1
                for c4 in range(0, nt, 4):
                    nb = min(4, nt - c4)
                    tp = trp.tile([P, 512], bf16, tag="tr", name="tp")
                    for kk in range(nb):
                        nc.tensor.transpose(tp[:, kk * P:(kk + 1) * P],
                                            Pt[:, (c4 + kk) * P:(c4 + kk + 1) * P],
                                            identb[:])
                    nc.any.tensor_copy(out=pt[:, c4 * P:(c4 + nb) * P],
                                       in_=tp[:, 0:nb * P])
                # unnormalized probs @ Vo; normalize by 1/Z at evacuation
                pv = pvp.tile([P, 512], f32, tag="pv", name="pv")
                for tb in range(nt):
                    nc.tensor.matmul(pv[:],
                                     pt[:, tb * P:(tb + 1) * P],
                                     Vo[h][tb][:],
                                     start=(tb == 0), stop=(tb == i))
                if h == 0:
                    nc.vector.tensor_scalar_mul(out=oacc[i][:], in0=pv[:],
                                                scalar1=zr[:])
                else:
                    of = up.tile([P, D], f32, tag="of", name="of")
                    nc.vector.tensor_scalar_mul(out=of[:], in0=pv[:],
                                                scalar1=zr[:])
                    of2 = up.tile([P, D], f16, tag="of2", name="of2")
                    nc.vector.tensor_add(out=of2[:], in0=of[:], in1=oacc[i][:])
                    dmaq[i % 2].dma_start(out=out_ext[i * P:(i + 1) * P, :],
                                          in_=of2[:])

            pend = [None]

            def emit_stage(h, i):
                cur = (h, i) + emit_scores(h, i)
                if pend[0] is not None:
                    emit_tail(*pend[0])
                pend[0] = cur

            # LN group g + B slice g + Vo tiles of the group interleave with
            # attention stages so projection matmuls fill softmax bubbles;
            # row-block 0 runs last to shrink the final drain.
            emit_ln_group(0)
            nc.gpsimd.dma_start(out=waT[:], in_=wa_ext[:, :])
            nc.scalar.dma_start(out=wgT[:], in_=wg_ext[:, :])
            nc.sync.dma_start(out=mask[:], in_=msk_ext[:, :])
            emit_b_sl(0, 0)
            emit_b_sl(1, 0)
            for t in range(4):
                emit_vo_tile(0, t)
                emit_vo_tile(1, t)
            stage_blocks = {1: (1,), 2: (2, 3), 3: (4, 5, 6, 7)}
            for g in (1, 2, 3):
                for i in stage_blocks[g]:
                    emit_stage(0, i)
                    emit_stage(1, i)
                emit_ln_group(g)
                emit_b_sl(0, g)
                emit_b_sl(1, g)
                for t in range(4 * g, 4 * g + 4):
                    emit_vo_tile(0, t)
                    emit_vo_tile(1, t)
            for i in range(8, NS):
                emit_stage(0, i)
                emit_stage(1, i)
            emit_stage(0, 0)
            emit_stage(1, 0)
            emit_tail(*pend[0], final=True)
    return nc


_NC = None


def _get_nc():
    global _NC
    if _NC is None:
        _NC = _build()
    return _NC


def _run(inputs, trace=False):
    x = np.asarray(inputs["x"], dtype=np.float32)          # [4, 2048, 512]
    gamma = np.asarray(inputs["gamma"], dtype=np.float32).reshape(D)
    beta = np.asarray(inputs["beta"], dtype=np.float32).reshape(D)
    Wq = np.asarray(inputs["Wq"], dtype=np.float32)        # [4, 512, 1024]
    Wk = np.asarray(inputs["Wk"], dtype=np.float32)
    Wv = np.asarray(inputs["Wv"], dtype=np.float32)
    Wout = np.asarray(inputs["Wout"], dtype=np.float32)    # [4096, 512]

    # softmax is invariant to row-constant shifts, so Q/K fold into
    # A_h = Wqf_h Wkf_h^T and the only surviving beta term is the
    # per-column vector ck_h = Wkf_h (beta Wq_h); V and the output
    # projection fold into G_h = Wvf_h Wout_h; the V-bias term passes
    # through softmax and is added host-side as cvec.
    Wqf = Wq * gamma[None, :, None]
    Wkf = Wk * gamma[None, :, None]
    Wvf = Wv * gamma[None, :, None]
    bq_all = np.einsum("d,hdu->hu", beta, Wq)              # [4, 1024]
    bv_all = np.einsum("d,hdu->hu", beta, Wv)              # [4, 1024]
    cvec = np.zeros(D, np.float32)
    A = np.zeros((4, D, D), np.float32)
    G = np.zeros((4, D, D), np.float32)
    ckv = np.zeros((4, D), np.float32)
    for h in range(4):
        A[h] = Wqf[h] @ Wkf[h].T
        G[h] = Wvf[h] @ Wout[h * U:(h + 1) * U]
        ckv[h] = Wkf[h] @ bq_all[h]
        cvec += bv_all[h] @ Wout[h * U:(h + 1) * U]

    # host-built constants: causal masks, identities
    pp = np.arange(P)[:, None]
    jj = np.arange(D)[None, :]
    msk = np.concatenate(
        [np.where(jj <= m * P + pp, 0.0, NEG) for m in range(4)],
        axis=1).astype(np.float16)                          # [128, 2048]
    idf = np.eye(P, dtype=np.float16)
    import ml_dtypes
    idb = np.eye(P, dtype=ml_dtypes.bfloat16)

    in_maps = []
    for c in range(8):
        b, hp = c // 2, c % 2
        ck = np.zeros((P, 2 * ND + 2), np.float32)
        ck[:, 0:2 * ND] = ckv[2 * hp:2 * hp + 2].reshape(2, ND, P).transpose(2, 0, 1).reshape(P, 2 * ND)
        ck[:, 2 * ND] = EPS
        ck[:, 2 * ND + 1] = EBIAS
        # pack weights as [128, (h*4+j)*512 + c]: partition = row within
        # 128-chunk, so one DMA loads all 8 stationary chunks per matrix
        wa = A[2 * hp:2 * hp + 2].reshape(2, ND, P, D).transpose(2, 0, 1, 3).reshape(P, 2 * ND * D)
        wg = G[2 * hp:2 * hp + 2].reshape(2, ND, P, D).transpose(2, 0, 1, 3).reshape(P, 2 * ND * D)
        in_maps.append({
            "x": np.ascontiguousarray(x[b]),
            "ck": np.ascontiguousarray(ck),
            "idf": idf,
            "idb": idb,
            "msk": msk,
            "wa": np.ascontiguousarray(wa).astype(np.float16),
            "wg": np.ascontiguousarray(wg).astype(np.float16),
        })
    res = run_bass_kernel_spmd(_get_nc(), in_maps, list(range(8)), trace=trace)
    out = np.empty((4, S, D), np.float32)
    for b in range(4):
        out[b] = (res.results[2 * b]["out"].astype(np.float32)
                  + res.results[2 * b + 1]["out"].astype(np.float32) + cvec[None, :])
    return out, res


def kernel(**inputs):
    out, _ = _run(inputs, trace=False)
    return out
